# revision 1
# baseline (speedup 1.0000x reference)
"""Data-parallel TRN2 kernel for nn_ConvEncodeDecodeLargeVQVAE.

Strategy (per sharding hint): pure data parallel — shard batch N=512 across the
8 NeuronCores (64 samples each), replicate all weights and the 512x32 codebook,
run the full encoder/VQ/decoder forward on each core, gather full outputs.
Implemented as a single jitted shard_map over the 8 axon-tunneled devices so the
whole forward compiles to one NEFF per core.
"""
import numpy as np
import jax
import jax.numpy as jnp
from jax import lax
from jax.sharding import Mesh, PartitionSpec as P
from functools import partial

EPS = 1e-5
N_CORES = 8

# ---- forward math (identical to the reference implementation) ----

def _conv(x, w, b, s, p):
    y = lax.conv_general_dilated(x, w, (s, s), [(p, p), (p, p)],
                                 dimension_numbers=('NCHW', 'OIHW', 'NCHW'))
    return y + b[None, :, None, None]


def _convT(x, w, b, s, p):
    wf = jnp.flip(w, (2, 3)).transpose(1, 0, 2, 3)
    q = w.shape[2] - 1 - p
    y = lax.conv_general_dilated(x, wf, (1, 1), [(q, q), (q, q)],
                                 lhs_dilation=(s, s),
                                 dimension_numbers=('NCHW', 'OIHW', 'NCHW'))
    return y + b[None, :, None, None]


def _bn(x, g, b, m, v):
    s = g / jnp.sqrt(v + EPS)
    return x * s[None, :, None, None] + (b - m * s)[None, :, None, None]


def _forward(x, ew1, eb1, bn1g, bn1b, bn1m, bn1v, ew2, eb2, bn2g, bn2b, bn2m, bn2v,
             ew3, eb3, bn3g, bn3b, bn3m, bn3v, ew4, eb4, bn4g, bn4b, bn4m, bn4v,
             fc21w, fc21b, fc22w, fc22b, fc3w, fc3b,
             cw1, cb1, cbn1g, cbn1b, cbn1m, cbn1v, cw2, cb2, emb,
             dw1, db1, dbn1g, dbn1b, dbn1m, dbn1v, dw2, db2, dbn2g, dbn2b, dbn2m, dbn2v,
             dw3, db3, dbn3g, dbn3b, dbn3m, dbn3v, dw4, db4, dbn4g, dbn4b, dbn4m, dbn4v,
             ow, ob):
    relu = jax.nn.relu
    N = x.shape[0]
    h = relu(_bn(_conv(x, ew1, eb1, 2, 1), bn1g, bn1b, bn1m, bn1v))
    h = relu(_bn(_conv(h, ew2, eb2, 2, 1), bn2g, bn2b, bn2m, bn2v))
    h = relu(_bn(_conv(h, ew3, eb3, 2, 1), bn3g, bn3b, bn3m, bn3v))
    h = relu(_bn(_conv(h, ew4, eb4, 1, 0), bn4g, bn4b, bn4m, bn4v))
    hl = h.reshape(N, -1)
    mu = hl @ fc21w + fc21b
    logvar = hl @ fc22w + fc22b
    z = mu
    co = relu(z @ fc3w + fc3b)
    col = co.reshape(N, 128, 9, 9)
    ze = relu(_bn(_convT(col, cw1, cb1, 1, 0), cbn1g, cbn1b, cbn1m, cbn1v))
    z_e_x = _convT(ze, cw2, cb2, 1, 0)
    zt = z_e_x.transpose(0, 2, 3, 1)
    d = ((zt * zt).sum(-1, keepdims=True)
         - 2.0 * jnp.einsum('nhwc,kc->nhwk', zt, emb)
         + (emb * emb).sum(-1))
    latents = jnp.argmin(d, axis=-1)
    z_q = emb[latents]
    z_q_x = z_q.transpose(0, 3, 1, 2)
    g = relu(_bn(_convT(z_q_x, dw1, db1, 1, 0), dbn1g, dbn1b, dbn1m, dbn1v))
    g = relu(_bn(_convT(g, dw2, db2, 2, 1), dbn2g, dbn2b, dbn2m, dbn2v))
    g = relu(_bn(_convT(g, dw3, db3, 2, 1), dbn3g, dbn3b, dbn3m, dbn3v))
    g = relu(_bn(_convT(g, dw4, db4, 2, 0), dbn4g, dbn4b, dbn4m, dbn4v))
    x_tilde = _convT(g, ow, ob, 1, 0)
    return (x_tilde, z, mu, logvar, z_e_x, z_q_x, latents)


_COMPILED = None


def _get_compiled():
    global _COMPILED
    if _COMPILED is not None:
        return _COMPILED
    devices = jax.devices()[:N_CORES]
    mesh = Mesh(np.asarray(devices), ("b",))

    def fwd(*args):
        return _forward(*args)

    # x is batch-sharded; all weights replicated; outputs batch-sharded.
    in_specs = (P("b"),) + (P(),) * 65
    out_specs = (P("b"),) * 7
    sharded = jax.jit(
        jax.shard_map(fwd, mesh=mesh, in_specs=in_specs, out_specs=out_specs,
                      check_vma=False)
        if hasattr(jax, "shard_map")
        else jax.experimental.shard_map.shard_map(  # older jax fallback
            fwd, mesh=mesh, in_specs=in_specs, out_specs=out_specs, check_rep=False)
    )
    _COMPILED = sharded
    return sharded


_ARG_ORDER = [
    'x', 'ew1', 'eb1', 'bn1g', 'bn1b', 'bn1m', 'bn1v', 'ew2', 'eb2', 'bn2g',
    'bn2b', 'bn2m', 'bn2v', 'ew3', 'eb3', 'bn3g', 'bn3b', 'bn3m', 'bn3v',
    'ew4', 'eb4', 'bn4g', 'bn4b', 'bn4m', 'bn4v', 'fc21w', 'fc21b', 'fc22w',
    'fc22b', 'fc3w', 'fc3b', 'cw1', 'cb1', 'cbn1g', 'cbn1b', 'cbn1m', 'cbn1v',
    'cw2', 'cb2', 'emb', 'dw1', 'db1', 'dbn1g', 'dbn1b', 'dbn1m', 'dbn1v',
    'dw2', 'db2', 'dbn2g', 'dbn2b', 'dbn2m', 'dbn2v', 'dw3', 'db3', 'dbn3g',
    'dbn3b', 'dbn3m', 'dbn3v', 'dw4', 'db4', 'dbn4g', 'dbn4b', 'dbn4m',
    'dbn4v', 'ow', 'ob',
]


def kernel(**inputs):
    fn = _get_compiled()
    args = [jnp.asarray(inputs[k]) for k in _ARG_ORDER]
    outs = fn(*args)
    outs = [np.asarray(o) for o in outs]
    # preserve dtypes: latents int32
    outs[6] = outs[6].astype(np.int32)
    return tuple(outs)


if __name__ == "__main__":
    import reference as R
    inputs = {k: np.asarray(v) for k, v in R.setup_inputs().items()}
    outs = kernel(**inputs)
    for o in outs:
        print(o.shape, o.dtype)
